# revision 40
# baseline (speedup 1.0000x reference)
"""Trainium2 Bass kernel for the NRI decoder (gnn_message_passing).

Strategy: data-parallel over batch B=8 across the 8 NeuronCores (one batch
item per core, params replicated; zero collectives).

Per-core algorithm (per recurrent step t, 9 steps):
  - fc1 of the edge MLP is factored through the nodes (exact, by
    associativity): pre @ W1 = rel_rec @ (hidden @ W1r) + rel_send @ (hidden @ W1s)
    so the heavy per-edge fc1 matmul collapses to two node-level matmuls
    (A = hidden@W1r, B = hidden@W1s) plus gather matmuls with rel_rec/rel_send.
  - gathers, fc2 and the scatter-aggregate are plain matmuls (no one-hot
    assumption anywhere), with the edge-type weights rtyp/(norm*d) folded into
    precomputed scatter weights wrec_k = rel_rec * rel_type[:, k] / 12 so the
    weighted sum over edge types becomes PSUM accumulation.
  - the scatter runs in fp8e4 with DoubleRow perf mode: chunk-pairs are
    fused into single 256-contraction matmuls (wrec pre-scaled x16 for fp8
    range; the 1/16 is folded into the GRU hid weights).
  - feature-major layouts chain all matmuls without transposes except one
    tiny [48,256] transpose of the aggregate per step.

Self-contained: hardcodes all shapes; no file reads.
"""

import numpy as np
import ml_dtypes

import concourse.tile as tile
from concourse import bacc, mybir
from concourse.bass import ts
from concourse.bass_utils import run_bass_kernel_spmd
from concourse.masks import make_identity

# Problem constants
B, N, T, D, H, K = 8, 48, 10, 4, 256, 4
E = N * (N - 1)          # 2256
NK = K - 1               # 3 used edge types (type 0 skipped)
TS = T - 1               # 9 recurrent steps
NORM = float(NK * D)     # combined 1/(K-1) and 1/n_in_node scaling
WREC_SCALE = 16.0        # fp8 range scaling for wrec; 1/16 folded into ghw

F32 = mybir.dt.float32
BF16 = mybir.dt.bfloat16
FP8 = mybir.dt.float8e4
AF = mybir.ActivationFunctionType
ALU = mybir.AluOpType
DR = mybir.MatmulPerfMode.DoubleRow

NEC128 = (E + 127) // 128                                   # 18
NPAIR = NEC128 // 2                                         # 9 chunk pairs
EC128 = [(i * 128, min(128, E - i * 128)) for i in range(NEC128)]
# tanh/scatter interleave parts for the last edge type, pair-aligned
TANH_PARTS = [(0, 8), (8, 16), (16, NEC128)]


def build_decoder(skip_t0: bool):
    nc = bacc.Bacc("TRN2", target_bir_lowering=False)

    d_data = nc.dram_tensor("data_fm", [5, T, N], F32, kind="ExternalInput")
    d_data_bf = nc.dram_tensor("data_bf", [5, T, N], BF16, kind="ExternalInput")
    d_relT = nc.dram_tensor("relT", [128, E], BF16, kind="ExternalInput")
    d_w1 = nc.dram_tensor("w1cat", [128, 2, NK, 2 * H], BF16, kind="ExternalInput")
    d_b1r = nc.dram_tensor("b1rows", [NK, H], BF16, kind="ExternalInput")
    d_w2 = nc.dram_tensor("w2", [128, 2, NK, H], BF16, kind="ExternalInput")
    d_b2 = nc.dram_tensor("b2bc", [128, NK, 2, H], BF16, kind="ExternalInput")
    d_wrec = nc.dram_tensor("wrec", [128, NK, NEC128, N], FP8, kind="ExternalInput")
    d_ghw = nc.dram_tensor("gru_hw", [128, 2, 3, H], BF16, kind="ExternalInput")
    d_giw = nc.dram_tensor("gru_iw", [5, 3, H], BF16, kind="ExternalInput")
    d_ow12 = nc.dram_tensor("outw12", [128, 2, 2, H], BF16, kind="ExternalInput")
    d_ob12 = nc.dram_tensor("outb12", [128, 2, 2], F32, kind="ExternalInput")
    d_o3w = nc.dram_tensor("out3w", [128, 2, D], BF16, kind="ExternalInput")
    d_o3b = nc.dram_tensor("out3b", [D, 1], F32, kind="ExternalInput")
    d_hid1 = None
    if skip_t0:
        d_hid1 = nc.dram_tensor("hid1", [128, 2, N], BF16, kind="ExternalInput")
    d_out = nc.dram_tensor("preds", [D, TS, N], F32, kind="ExternalOutput")

    with tile.TileContext(nc) as tc:
        with (
            tc.tile_pool(name="const", bufs=1) as cpool,
            tc.tile_pool(name="state", bufs=1) as spool,
            tc.tile_pool(name="work", bufs=3) as wpool,
            tc.tile_pool(name="stage", bufs=1) as zpool,
            tc.tile_pool(name="pA", bufs=1, space="PSUM") as pA,
            tc.tile_pool(name="pB", bufs=3, space="PSUM") as pB,
            tc.tile_pool(name="pC", bufs=1, space="PSUM") as pC,
        ):
            # ---------------- constants ----------------
            # Startup DMAs split across the three queues in consumption
            # order, pieced so the first gather can start ~3us in:
            #   sync:   hidden, w1 per edge type (fc1 path)
            #   scalar: relT in column pieces, then w2 (gather/fc2 path)
            #   gpsimd: everything else
            hidden = spool.tile([128, 2, N], BF16)
            if skip_t0:
                nc.sync.dma_start(hidden[:], d_hid1[:])
            else:
                nc.vector.memset(hidden[:], 0.0)
            w1 = cpool.tile([128, 2, NK, 2 * H], BF16)
            for k in range(NK):
                nc.sync.dma_start(w1[:, :, k, :], d_w1[:, :, k, :])
            b2bc = cpool.tile([128, NK, 2, H], BF16)
            nc.sync.dma_start(b2bc[:], d_b2[:])
            relT = cpool.tile([128, E], BF16)
            for e0, e1 in ((0, 512), (512, 1024), (1024, 2048), (2048, E)):
                nc.scalar.dma_start(relT[:, e0:e1], d_relT[:, e0:e1])
            w2 = cpool.tile([128, 2, NK, H], BF16)
            nc.scalar.dma_start(w2[:], d_w2[:])
            wrec = cpool.tile([128, NK, NEC128, N], FP8)
            nc.scalar.dma_start(wrec[:], d_wrec[:])
            ins5b = spool.tile([5, T, N], BF16)
            nc.gpsimd.dma_start(ins5b[:], d_data_bf[:])
            ghw = cpool.tile([128, 2, 3, H], BF16)
            nc.gpsimd.dma_start(ghw[:], d_ghw[:])
            giw = cpool.tile([5, 3, H], BF16)
            nc.gpsimd.dma_start(giw[:], d_giw[:])
            ow12 = cpool.tile([128, 2, 2, H], BF16)
            nc.gpsimd.dma_start(ow12[:], d_ow12[:])
            ob12 = cpool.tile([128, 2, 2], F32)
            nc.gpsimd.dma_start(ob12[:], d_ob12[:])
            o3w = cpool.tile([128, 2, D], BF16)
            nc.gpsimd.dma_start(o3w[:], d_o3w[:])
            o3b = cpool.tile([D, 1], F32)
            nc.gpsimd.dma_start(o3b[:], d_o3b[:])
            ins5 = spool.tile([5, T, N], F32)
            nc.gpsimd.dma_start(ins5[:], d_data[:])
            ident = cpool.tile([128, 128], F32)
            make_identity(nc, ident[:])

            # ---------------- state ----------------
            agg = spool.tile([128, 2, N], BF16)
            nc.vector.memset(agg[:], 0.0)
            preds = spool.tile([D, TS, N], F32)
            m1 = spool.tile([128, 2, NK, E], BF16)
            # AB_k: rows 0:48 = A = hidden@W1r (node-major), rows 64:112 = B.
            # Rows 48:64 / 112:128 stay zero forever (they hit zero rows of relT).
            ABs = []
            for k in range(NK):
                ab = spool.tile([128, H], BF16, tag=f"AB{k}")
                nc.vector.memset(ab[:], 0.0)
                nc.sync.dma_start(ab[48:49, :], d_b1r[k : k + 1, :])
                ABs.append(ab)
            # z2/m2 strips live across steps; the pad rows of the last chunk
            # are zeroed once so full-strip tanh + fp8 scatter read clean data.
            z2s = [
                zpool.tile([128, NEC128, H], F32, tag=f"z2_{k}", name=f"z2_{k}")
                for k in range(NK)
            ]
            m2s = [
                zpool.tile([128, NEC128, H], FP8, tag=f"m2_{k}", name=f"m2_{k}")
                for k in range(NK)
            ]
            # pad-zero the last chunk's tail rows (base partition must be
            # 32-aligned; rows 64:80 get overwritten by real data each step)
            for k in range(NK):
                nc.vector.memset(z2s[k][64:128, NEC128 - 1, :], 0.0)

            def node_fc1(k):
                ps = pB.tile([N, 2 * H], F32, tag="ps")
                for fc in range(2):
                    nc.tensor.matmul(
                        ps[:],
                        hidden[:, fc, :],
                        w1[:, fc, k, :],
                        start=(fc == 0),
                        stop=(fc == 1),
                    )
                # col-half-split copies so the first gather LDW (cols 0:128
                # of both A and B) unblocks half a copy earlier
                for c in range(2):
                    nc.vector.tensor_copy(
                        ABs[k][0:N, ts(c, 128)], ps[:, ts(c, 128)]
                    )
                    nc.vector.tensor_copy(
                        ABs[k][64 : 64 + N, ts(c, 128)], ps[:, H + 128 * c : H + 128 * (c + 1)]
                    )

            def gather_blk(k, blk, half):
                # m1[h', e] = tanh(A[recv] + B[send] + b1): one matmul on the
                # stacked [A;B] / [rel_rec.T; rel_send.T] operands; the fc1
                # bias rides contract-row 48 (relT row 48 is all-ones, AB row
                # 48 holds b1), so the tanh needs no bias operand.
                e0 = blk * 1024
                ps = pA.tile([128, 1024], F32, tag="gather", bufs=2)
                for c0 in range(0, 1024, 512):
                    nc.tensor.matmul(
                        ps[:, c0 : c0 + 512],
                        ABs[k][:, ts(half, 128)],
                        relT[:, e0 + c0 : e0 + c0 + 512],
                    )
                nc.scalar.activation(
                    m1[:, half, k, e0 : e0 + 1024], ps[:], AF.Tanh
                )

            def gather_tail(k):
                # both halves' 208-col tails share one psum + one ACT
                pt = pA.tile([128, 2, E - 2048], F32, tag="gather", name="ptail", bufs=2)
                for half in range(2):
                    nc.tensor.matmul(
                        pt[:, half, :],
                        ABs[k][:, ts(half, 128)],
                        relT[:, 2048:E],
                    )
                nc.scalar.activation(m1[:, 0:2, k, 2048:E], pt[:], AF.Tanh)

            def fc2(k, z2, groups=(0, NEC128)):
                for g0 in range(groups[0], groups[1], 2):
                    sub = EC128[g0 : g0 + 2]
                    ps = pB.tile([128, 2, H], F32, tag="ps")
                    for j, (e0, ew) in enumerate(sub):
                        for fc in range(2):
                            nc.tensor.matmul(
                                ps[:ew, j, :],
                                m1[:, fc, k, e0 : e0 + ew],
                                w2[:, fc, k, :],
                                start=(fc == 0),
                                stop=(fc == 1),
                            )
                    if sub[-1][1] == 128:
                        nc.vector.tensor_tensor(
                            z2[:, g0 : g0 + 2, :], ps[:], b2bc[:, k, :, :], ALU.add
                        )
                    else:
                        for j, (e0, ew) in enumerate(sub):
                            nc.vector.tensor_tensor(
                                z2[:ew, g0 + j, :],
                                ps[:ew, j, :],
                                b2bc[:ew, k, j, :],
                                ALU.add,
                            )

            def mega_tanh(k, parts=((0, NEC128),)):
                # full [128, chunks, H] strips; pad rows of the last chunk are
                # zeroed once at build, so tanh over them is clean (m2 pad
                # rows become 0, required by the fp8 scatter pairs).
                for c0, c1 in parts:
                    nc.scalar.activation(
                        m2s[k][:, c0:c1, :], z2s[k][:, c0:c1, :], AF.Tanh
                    )

            def scatter(k, agg_ps, first, last, chunks=(0, NEC128)):
                # single accumulation into rows 0:48 (all-fp8 runs at bf16
                # speed); no col packing so the aggregate drains with one
                # copy at the step boundary
                for ci in range(chunks[0], chunks[1]):
                    e0, ew = EC128[ci]
                    nc.tensor.matmul(
                        agg_ps[0:N, :],
                        wrec[:ew, k, ci, :],
                        m2s[k][:ew, ci, :],
                        start=(first and ci == 0),
                        stop=(last and ci == NEC128 - 1),
                        skip_group_check=True,
                    )

            def ka_warm():
                # HAM warmth filler: the GRU boundary leaves the PE mostly
                # idle for ~4us, which re-throttles the PE clock to 1.2 GHz
                # and makes every step's first ~3.4us of matmuls run at half
                # speed. These N=512 reads of constant w2 keep the activity
                # monitor busy through the boundary.
                kaw = pA.tile([128, 512], F32, tag="gather", name="kawarm", bufs=2)
                nc.tensor.matmul(kaw[0:2, :], w1[:, 0, 0, 0:2], w1[:, 0, 0, :])

            def edge_phase(pending_out=None):
                # Engine queues execute in order, so emission order IS the
                # schedule. The merge below rate-matches the PE against the
                # ACT m1-tanh stream: gathers of type k+1 and fc2 groups of
                # type k alternate at psum-tile granularity, fc2 of the last
                # type runs before the (mega-tanh-gated) scatters.
                node_fc1(0)
                node_fc1(1)
                node_fc1(2)
                gather_blk(0, 0, 0)
                gather_blk(0, 0, 1)
                if pending_out is not None:
                    pending_out()  # deferred out-MLP: filler during G0 pacing
                gather_blk(0, 1, 0)
                gather_blk(0, 1, 1)
                gather_tail(0)
                # fine-grained merge: one gather psum tile then one fc2 pair
                # group, so the PE returns to refill the m1-tanh pipe quickly
                # and the ACT stream never starves
                gather_blk(1, 0, 0)
                fc2(0, z2s[0], (0, 2))
                gather_blk(1, 0, 1)
                fc2(0, z2s[0], (2, 4))
                gather_blk(1, 1, 0)
                fc2(0, z2s[0], (4, 6))
                gather_blk(1, 1, 1)
                fc2(0, z2s[0], (6, 8))
                gather_tail(1)
                fc2(0, z2s[0], (8, 12))
                gather_blk(2, 0, 0)
                fc2(0, z2s[0], (12, 16))
                gather_blk(2, 0, 1)
                fc2(0, z2s[0], (16, NEC128))
                gather_blk(2, 1, 0)
                fc2(1, z2s[1], (0, 2))
                gather_blk(2, 1, 1)
                fc2(1, z2s[1], (2, 4))
                gather_tail(2)
                fc2(1, z2s[1], (4, 8))
                fc2(1, z2s[1], (8, 12))
                fc2(1, z2s[1], (12, 16))
                fc2(1, z2s[1], (16, NEC128))
                mega_tanh(0)
                fc2(2, z2s[2], (0, 8))
                fc2(2, z2s[2], (8, 16))
                fc2(2, z2s[2], (16, NEC128))
                agg_ps = pC.tile([128, H], F32, tag="agg")
                scatter(0, agg_ps, True, False)
                mega_tanh(1)
                scatter(1, agg_ps, False, False)
                # last type is ACT-bound: interleave tanh parts with scatter
                # parts; last=True for every part since stop tags the final
                # accumulation MM of each col group (both in the final part)
                for c0, c1 in TANH_PARTS:
                    mega_tanh(2, parts=((c0, c1),))
                    scatter(2, agg_ps, False, True, chunks=(c0, c1))

                # aggregate: merge the two col groups and transpose to
                # feature-major, split per column half to shorten the chain
                agg_nm = wpool.tile([N, H], F32, tag="aggnm")
                nc.vector.tensor_copy(agg_nm[:], agg_ps[0:N, :])
                for half in range(2):
                    ka_warm()
                    tp = pB.tile([128, N], F32, tag="ps")
                    nc.tensor.transpose(tp[:], agg_nm[:, ts(half, 128)], ident[:N, :N])
                    nc.vector.tensor_copy(agg[:, half, :], tp[:])

            def gru_and_out(t):
                insT = ins5b[:, t, :]  # [5, 48]; row 4 is ones (folds input biases)
                # r gate first: it sits on the nng critical path; i only
                # matters at the final mix
                ps_r = pB.tile([128, 2, N], F32, tag="ps")
                for half in range(2):
                    nc.tensor.matmul(
                        ps_r[:, half, :], giw[:, 0, ts(half, 128)], insT,
                        start=True, stop=False,
                    )
                    for fc in range(2):
                        nc.tensor.matmul(
                            ps_r[:, half, :],
                            ghw[:, fc, 0, ts(half, 128)],
                            agg[:, fc, :],
                            start=False,
                            stop=(fc == 1),
                        )
                r_fm = wpool.tile([128, 2, N], F32, tag="gate0")
                nc.scalar.activation(r_fm[:], ps_r[:], AF.Sigmoid)
                ka_warm()
                ps_hn = pB.tile([128, 2, N], F32, tag="ps")
                ps_in = pB.tile([128, 2, N], F32, tag="ps")
                for half in range(2):
                    for fc in range(2):
                        nc.tensor.matmul(
                            ps_hn[:, half, :],
                            ghw[:, fc, 2, ts(half, 128)],
                            agg[:, fc, :],
                            start=(fc == 0),
                            stop=(fc == 1),
                        )
                    nc.tensor.matmul(
                        ps_in[:, half, :], giw[:, 2, ts(half, 128)], insT
                    )
                ka_warm()
                ps_i = pB.tile([128, 2, N], F32, tag="ps")
                for half in range(2):
                    nc.tensor.matmul(
                        ps_i[:, half, :], giw[:, 1, ts(half, 128)], insT,
                        start=True, stop=False,
                    )
                    for fc in range(2):
                        nc.tensor.matmul(
                            ps_i[:, half, :],
                            ghw[:, fc, 1, ts(half, 128)],
                            agg[:, fc, :],
                            start=False,
                            stop=(fc == 1),
                        )
                ka_warm()
                ka = pA.tile([128, N], F32, tag="gather", name="keepalive", bufs=2)
                nc.tensor.matmul(ka[0:2, :], r_fm[:, 0, 0:2], r_fm[:, 0, :])
                t1 = wpool.tile([128, 2, N], F32, tag="t1")
                nng = wpool.tile([128, 2, N], F32, tag="nng")
                dlt = wpool.tile([128, 2, N], F32, tag="dlt")
                i_fm = wpool.tile([128, 2, N], F32, tag="gate1")
                # ACT queue order matters: nng(h0) tanh is emitted BEFORE the
                # i-sigmoid so the critical path to hidden[h0] (and the next
                # step's fc1) doesn't wait behind it; both t1 halves go to
                # the DVE first so nng(h1) can fire right after.
                for h in range(2):
                    nc.vector.tensor_mul(t1[:, h, :], r_fm[:, h, :], ps_hn[:, h, :])
                    nc.vector.tensor_add(t1[:, h, :], t1[:, h, :], ps_in[:, h, :])
                nc.scalar.activation(nng[:, 0, :], t1[:, 0, :], AF.Tanh)
                nc.scalar.activation(i_fm[:], ps_i[:], AF.Sigmoid)
                nc.scalar.activation(nng[:, 1, :], t1[:, 1, :], AF.Tanh)
                # hidden = (1-i)*nng + i*hidden = nng + i*(hidden-nng),
                # per feature-half so hidden[:, h] lands early and the next
                # step's node-fc1 matmul can start sooner
                for h in range(2):
                    nc.vector.tensor_sub(dlt[:, h, :], hidden[:, h, :], nng[:, h, :])
                    nc.vector.tensor_mul(dlt[:, h, :], i_fm[:, h, :], dlt[:, h, :])
                    nc.vector.tensor_add(hidden[:, h, :], nng[:, h, :], dlt[:, h, :])
                nc.tensor.matmul(ka[0:2, :], nng[:, 0, 0:2], nng[:, 0, :])
                ka_warm()
                ka_warm()

                def out_mlp(t=t):
                    emit_out_mlp(t)
                return out_mlp

            def emit_out_mlp(t):
                # output MLP with residual
                cur = hidden
                for layer in range(2):
                    ps = pB.tile([128, 2, N], F32, tag="ps")
                    for half in range(2):
                        for fc in range(2):
                            nc.tensor.matmul(
                                ps[:, half, :],
                                ow12[:, fc, layer, ts(half, 128)],
                                cur[:, fc, :],
                                start=(fc == 0),
                                stop=(fc == 1),
                            )
                    nxt = wpool.tile([128, 2, N], BF16, tag=f"p{layer}")
                    for half in range(2):
                        nc.vector.tensor_scalar(
                            nxt[:, half, :],
                            ps[:, half, :],
                            ob12[:, half, layer : layer + 1],
                            0.0,
                            ALU.add,
                            ALU.max,
                        )
                    cur = nxt
                ps3 = pB.tile([D, N], F32, tag="ps")
                for fc in range(2):
                    nc.tensor.matmul(
                        ps3[:],
                        o3w[:, fc, :],
                        cur[:, fc, :],
                        start=(fc == 0),
                        stop=(fc == 1),
                    )
                # pred = (ps3 + b3) + ins
                nc.vector.scalar_tensor_tensor(
                    preds[:, t, :], ps3[:], o3b[:], ins5[0:D, t, :], ALU.add, ALU.add
                )
                nc.sync.dma_start(d_out[:, t, :], preds[:, t, :])

            pending_out = None
            for t in range(1 if skip_t0 else 0, TS):
                edge_phase(pending_out)
                pending_out = gru_and_out(t)
            pending_out()

    return nc


def _prep_core(b: int, inputs: dict) -> dict:
    f32 = np.float32
    bf16 = ml_dtypes.bfloat16
    fp8 = ml_dtypes.float8_e4m3
    data = np.asarray(inputs["data"], f32)
    rel_type = np.asarray(inputs["rel_type"], f32)
    rel_rec = np.asarray(inputs["rel_rec"], f32)
    rel_send = np.asarray(inputs["rel_send"], f32)
    w1 = np.asarray(inputs["msg_fc1_w"], f32)
    b1 = np.asarray(inputs["msg_fc1_b"], f32)
    w2 = np.asarray(inputs["msg_fc2_w"], f32)
    b2 = np.asarray(inputs["msg_fc2_b"], f32)

    m = {}
    dfm = np.ones((5, T, N), f32)
    dfm[0:4] = data[b].transpose(2, 1, 0)  # [N,T,D] -> [D,T,N]
    m["data_fm"] = dfm
    m["data_bf"] = dfm.astype(bf16)

    relT = np.zeros((128, E), f32)
    relT[0:N] = rel_rec.T
    relT[48] = 1.0  # bias row: pairs with AB row 48 = msg_fc1_b
    relT[64 : 64 + N] = rel_send.T
    m["relT"] = relT.astype(bf16)

    w1c = np.zeros((128, 2, NK, 2 * H), f32)
    for k in range(NK):
        wk = w1[k + 1]  # [2H, H]
        cat = np.concatenate([wk[:H], wk[H:]], axis=1)  # [H, 2H] = [W1r | W1s]
        w1c[:, :, k, :] = cat.reshape(2, 128, 2 * H).transpose(1, 0, 2)
    m["w1cat"] = w1c.astype(bf16)
    m["b1rows"] = np.stack([b1[k + 1] for k in range(NK)], axis=0).astype(bf16)

    m["w2"] = np.stack(
        [w2[k + 1].reshape(2, 128, H).transpose(1, 0, 2) for k in range(NK)], axis=2
    ).astype(bf16)
    b2bc = np.zeros((128, NK, 2, H), f32)
    for k in range(NK):
        b2bc[:, k, :, :] = b2[k + 1][None, None, :]
    m["b2bc"] = b2bc.astype(bf16)

    # wrec: [128, NK, NEC128, N] fp8, scaled x16 (1/16 folded into ghw)
    wr = np.zeros((128, NK, NEC128, N), f32)
    for k in range(NK):
        wk = rel_rec * rel_type[b, :, k + 1 : k + 2] * (WREC_SCALE / NORM)  # [E, N]
        wkp = np.zeros((NEC128 * 128, N), f32)
        wkp[:E] = wk
        wr[:, k] = wkp.reshape(NEC128, 128, N).transpose(1, 0, 2)
    m["wrec"] = wr.astype(fp8)

    m["gru_hw"] = (
        np.stack(
            [
                np.asarray(inputs[n], f32).reshape(2, 128, H).transpose(1, 0, 2)
                for n in ["hid_r_w", "hid_i_w", "hid_n_w"]
            ],
            axis=2,
        )
        / WREC_SCALE
    ).astype(bf16)
    giw = np.zeros((5, 3, H), f32)
    for g, (wn, bn) in enumerate(
        [("in_r_w", "in_r_b"), ("in_i_w", "in_i_b"), ("in_n_w", "in_n_b")]
    ):
        giw[0:4, g] = np.asarray(inputs[wn], f32)
        giw[4, g] = np.asarray(inputs[bn], f32)
    m["gru_iw"] = giw.astype(bf16)

    m["outw12"] = np.stack(
        [
            np.asarray(inputs[n], f32).reshape(2, 128, H).transpose(1, 0, 2)
            for n in ["out1_w", "out2_w"]
        ],
        axis=2,
    ).astype(bf16)
    m["outb12"] = np.stack(
        [np.asarray(inputs[n], f32).reshape(2, 128).T for n in ["out1_b", "out2_b"]],
        axis=2,
    )
    m["out3w"] = np.asarray(inputs["out3_w"], f32).reshape(2, 128, D).transpose(1, 0, 2).astype(bf16)
    m["out3b"] = np.asarray(inputs["out3_b"], f32).reshape(D, 1)
    return m


def _skip_t0_ok(inputs) -> bool:
    # With hidden0 == 0, the whole edge phase at t=0 yields agg == 0 iff the
    # message-MLP biases of the used edge types are zero.
    return bool(
        np.all(np.asarray(inputs["msg_fc1_b"])[1:] == 0)
        and np.all(np.asarray(inputs["msg_fc2_b"])[1:] == 0)
    )


def _host_t0(inputs):
    """Step 0 in exact fp32 on the host: hidden0 == 0 and zero message-MLP
    biases make agg(t=0) == 0, so only the GRU + output MLP run."""
    f32 = np.float32
    data = np.asarray(inputs["data"], f32)
    ins0 = data[:, :, 0, :]  # [B, N, D]
    sig = lambda x: 1.0 / (1.0 + np.exp(-x))
    i = sig(ins0 @ np.asarray(inputs["in_i_w"], f32) + np.asarray(inputs["in_i_b"], f32))
    n = np.tanh(ins0 @ np.asarray(inputs["in_n_w"], f32) + np.asarray(inputs["in_n_b"], f32))
    hidden1 = (1.0 - i) * n  # [B, N, H]
    p = np.maximum(hidden1 @ np.asarray(inputs["out1_w"], f32) + np.asarray(inputs["out1_b"], f32), 0)
    p = np.maximum(p @ np.asarray(inputs["out2_w"], f32) + np.asarray(inputs["out2_b"], f32), 0)
    pred0 = ins0 + p @ np.asarray(inputs["out3_w"], f32) + np.asarray(inputs["out3_b"], f32)
    return hidden1, pred0


def prepare(inputs):
    """Returns (skip_t0, in_maps, pred0) — shared by kernel() and harnesses."""
    skip_t0 = _skip_t0_ok(inputs)
    in_maps = [_prep_core(b, inputs) for b in range(B)]
    pred0 = None
    if skip_t0:
        hidden1, pred0 = _host_t0(inputs)
        for b in range(B):
            # hidden feature-major [128, 2, N]: [p, half, node] = hidden1[node, half*128+p]
            in_maps[b]["hid1"] = np.ascontiguousarray(
                hidden1[b].T.reshape(2, 128, N).transpose(1, 0, 2)
            ).astype(ml_dtypes.bfloat16)
    return skip_t0, in_maps, pred0


def kernel(**inputs) -> np.ndarray:
    assert int(inputs["pred_steps"]) == 1
    skip_t0, in_maps, pred0 = prepare(inputs)
    nc = build_decoder(skip_t0)
    nc.compile()
    res = run_bass_kernel_spmd(nc, in_maps, core_ids=list(range(B)))
    out = np.stack(
        [res.results[b]["preds"].transpose(2, 1, 0) for b in range(B)], axis=0
    )
    out = out.astype(np.float32)
    if skip_t0:
        out[:, :, 0, :] = pred0
    return out


if __name__ == "__main__":
    # smoke: build only
    nc = build_decoder(True)
    print("built ok")


# revision 43
# speedup vs baseline: 1.0155x; 1.0155x over previous
"""Trainium2 Bass kernel for the NRI decoder (gnn_message_passing).

Strategy: data-parallel over batch B=8 across the 8 NeuronCores (one batch
item per core, params replicated; zero collectives).

Per-core algorithm (per recurrent step t, 9 steps):
  - fc1 of the edge MLP is factored through the nodes (exact, by
    associativity): pre @ W1 = rel_rec @ (hidden @ W1r) + rel_send @ (hidden @ W1s)
    so the heavy per-edge fc1 matmul collapses to two node-level matmuls
    (A = hidden@W1r, B = hidden@W1s) plus gather matmuls with rel_rec/rel_send.
  - gathers, fc2 and the scatter-aggregate are plain matmuls (no one-hot
    assumption anywhere), with the edge-type weights rtyp/(norm*d) folded into
    precomputed scatter weights wrec_k = rel_rec * rel_type[:, k] / 12 so the
    weighted sum over edge types becomes PSUM accumulation.
  - the scatter runs in fp8e4 with DoubleRow perf mode: chunk-pairs are
    fused into single 256-contraction matmuls (wrec pre-scaled x16 for fp8
    range; the 1/16 is folded into the GRU hid weights).
  - feature-major layouts chain all matmuls without transposes except one
    tiny [48,256] transpose of the aggregate per step.

Self-contained: hardcodes all shapes; no file reads.
"""

import numpy as np
import ml_dtypes

import concourse.tile as tile
from concourse import bacc, mybir
from concourse.bass import ts
from concourse.bass_utils import run_bass_kernel_spmd
from concourse.masks import make_identity

# Problem constants
B, N, T, D, H, K = 8, 48, 10, 4, 256, 4
E = N * (N - 1)          # 2256
NK = K - 1               # 3 used edge types (type 0 skipped)
TS = T - 1               # 9 recurrent steps
NORM = float(NK * D)     # combined 1/(K-1) and 1/n_in_node scaling
WREC_SCALE = 16.0        # fp8 range scaling for wrec; 1/16 folded into ghw

F32 = mybir.dt.float32
BF16 = mybir.dt.bfloat16
FP8 = mybir.dt.float8e4
AF = mybir.ActivationFunctionType
ALU = mybir.AluOpType
DR = mybir.MatmulPerfMode.DoubleRow

NEC128 = (E + 127) // 128                                   # 18
NPAIR = NEC128 // 2                                         # 9 chunk pairs
EC128 = [(i * 128, min(128, E - i * 128)) for i in range(NEC128)]
# tanh/scatter interleave parts for the last edge type, pair-aligned
TANH_PARTS = [(0, 8), (8, 16), (16, NEC128)]


def build_decoder(skip_t0: bool):
    nc = bacc.Bacc("TRN2", target_bir_lowering=False)

    d_data = nc.dram_tensor("data_fm", [5, T, N], F32, kind="ExternalInput")
    d_data_bf = nc.dram_tensor("data_bf", [5, T, N], BF16, kind="ExternalInput")
    d_relT = nc.dram_tensor("relT", [128, E], BF16, kind="ExternalInput")
    d_w1 = nc.dram_tensor("w1cat", [128, 2, NK, 2 * H], BF16, kind="ExternalInput")
    d_b1r = nc.dram_tensor("b1rows", [NK, H], BF16, kind="ExternalInput")
    d_w2 = nc.dram_tensor("w2", [128, 2, NK, H], BF16, kind="ExternalInput")
    d_b2 = nc.dram_tensor("b2bc", [128, NK, 2, H], BF16, kind="ExternalInput")
    d_wrec = nc.dram_tensor("wrec", [128, NK, NEC128, N], FP8, kind="ExternalInput")
    d_ghw = nc.dram_tensor("gru_hw", [128, 2, 3, H], BF16, kind="ExternalInput")
    d_giw = nc.dram_tensor("gru_iw", [5, 3, H], BF16, kind="ExternalInput")
    d_ow12 = nc.dram_tensor("outw12", [128, 2, 2, H], BF16, kind="ExternalInput")
    d_ob12 = nc.dram_tensor("outb12", [128, 2, 2], F32, kind="ExternalInput")
    d_o3w = nc.dram_tensor("out3w", [128, 2, D], BF16, kind="ExternalInput")
    d_o3b = nc.dram_tensor("out3b", [D, 1], F32, kind="ExternalInput")
    d_hid1 = None
    if skip_t0:
        d_hid1 = nc.dram_tensor("hid1", [128, 2, N], BF16, kind="ExternalInput")
    d_out = nc.dram_tensor("preds", [D, TS, N], F32, kind="ExternalOutput")

    with tile.TileContext(nc) as tc:
        with (
            tc.tile_pool(name="const", bufs=1) as cpool,
            tc.tile_pool(name="state", bufs=1) as spool,
            tc.tile_pool(name="work", bufs=3) as wpool,
            tc.tile_pool(name="stage", bufs=1) as zpool,
            tc.tile_pool(name="pA", bufs=1, space="PSUM") as pA,
            tc.tile_pool(name="pB", bufs=2, space="PSUM") as pB,
        ):
            # ---------------- constants ----------------
            # Startup DMAs split across the three queues in consumption
            # order, pieced so the first gather can start ~3us in:
            #   sync:   hidden, w1 per edge type (fc1 path)
            #   scalar: relT in column pieces, then w2 (gather/fc2 path)
            #   gpsimd: everything else
            hidden = spool.tile([128, 2, N], BF16)
            if skip_t0:
                nc.sync.dma_start(hidden[:], d_hid1[:])
            else:
                nc.vector.memset(hidden[:], 0.0)
            w1 = cpool.tile([128, 2, NK, 2 * H], BF16)
            for k in range(NK):
                nc.sync.dma_start(w1[:, :, k, :], d_w1[:, :, k, :])
            b2bc = cpool.tile([128, NK, 2, H], BF16)
            nc.sync.dma_start(b2bc[:], d_b2[:])
            relT = cpool.tile([128, E], BF16)
            for e0, e1 in ((0, 512), (512, 1024), (1024, 2048), (2048, E)):
                nc.scalar.dma_start(relT[:, e0:e1], d_relT[:, e0:e1])
            w2 = cpool.tile([128, 2, NK, H], BF16)
            nc.scalar.dma_start(w2[:], d_w2[:])
            wrec = cpool.tile([128, NK, NEC128, N], FP8)
            nc.scalar.dma_start(wrec[:], d_wrec[:])
            ins5b = spool.tile([5, T, N], BF16)
            nc.gpsimd.dma_start(ins5b[:], d_data_bf[:])
            ghw = cpool.tile([128, 2, 3, H], BF16)
            nc.gpsimd.dma_start(ghw[:], d_ghw[:])
            giw = cpool.tile([5, 3, H], BF16)
            nc.gpsimd.dma_start(giw[:], d_giw[:])
            ow12 = cpool.tile([128, 2, 2, H], BF16)
            nc.gpsimd.dma_start(ow12[:], d_ow12[:])
            ob12 = cpool.tile([128, 2, 2], F32)
            nc.gpsimd.dma_start(ob12[:], d_ob12[:])
            o3w = cpool.tile([128, 2, D], BF16)
            nc.gpsimd.dma_start(o3w[:], d_o3w[:])
            o3b = cpool.tile([D, 1], F32)
            nc.gpsimd.dma_start(o3b[:], d_o3b[:])
            ins5 = spool.tile([5, T, N], F32)
            nc.gpsimd.dma_start(ins5[:], d_data[:])
            ident = cpool.tile([128, 128], F32)
            make_identity(nc, ident[:])

            # ---------------- state ----------------
            agg = spool.tile([128, 2, N], BF16)
            nc.vector.memset(agg[:], 0.0)
            preds = spool.tile([D, TS, N], F32)
            m1 = spool.tile([128, 2, NK, E], BF16)
            # AB_k: rows 0:48 = A = hidden@W1r (node-major), rows 64:112 = B.
            # Rows 48:64 / 112:128 stay zero forever (they hit zero rows of relT).
            ABs = []
            for k in range(NK):
                ab = spool.tile([128, H], BF16, tag=f"AB{k}")
                nc.vector.memset(ab[:], 0.0)
                nc.sync.dma_start(ab[48:49, :], d_b1r[k : k + 1, :])
                ABs.append(ab)
            # z2/m2 strips live across steps; the pad rows of the last chunk
            # are zeroed once so full-strip tanh + fp8 scatter read clean data.
            z2s = [
                zpool.tile([128, NEC128, H], F32, tag=f"z2_{k}", name=f"z2_{k}")
                for k in range(NK)
            ]
            m2s = [
                zpool.tile([128, NEC128, H], FP8, tag=f"m2_{k}", name=f"m2_{k}")
                for k in range(NK)
            ]
            # pad-zero the last chunk's tail rows (base partition must be
            # 32-aligned; rows 64:80 get overwritten by real data each step)
            for k in range(NK):
                nc.vector.memset(z2s[k][64:128, NEC128 - 1, :], 0.0)

            def node_fc1(k):
                ps = pB.tile([N, 2 * H], F32, tag="ps")
                for fc in range(2):
                    nc.tensor.matmul(
                        ps[:],
                        hidden[:, fc, :],
                        w1[:, fc, k, :],
                        start=(fc == 0),
                        stop=(fc == 1),
                    )
                # col-half-split copies so the first gather LDW (cols 0:128
                # of both A and B) unblocks half a copy earlier
                for c in range(2):
                    nc.vector.tensor_copy(
                        ABs[k][0:N, ts(c, 128)], ps[:, ts(c, 128)]
                    )
                    nc.vector.tensor_copy(
                        ABs[k][64 : 64 + N, ts(c, 128)], ps[:, H + 128 * c : H + 128 * (c + 1)]
                    )

            def gather_blk(blk, half):
                # m1[h', e] = tanh(A[recv] + B[send] + b1): one matmul on the
                # stacked [A;B] / [rel_rec.T; rel_send.T] operands; the fc1
                # bias rides contract-row 48 (relT row 48 is all-ones, AB row
                # 48 holds b1), so the tanh needs no bias operand.
                # All THREE edge types share one [128, 3, 512] psum tile so a
                # single ACT call covers 1536 columns (amortizes the ~352-cyc
                # ACT call overhead 3x better).
                e0 = blk * 512
                ps = pA.tile([128, NK, 512], F32, tag="gather", bufs=2)
                for k in range(NK):
                    nc.tensor.matmul(
                        ps[:, k, :],
                        ABs[k][:, ts(half, 128)],
                        relT[:, e0 : e0 + 512],
                    )
                nc.scalar.activation(
                    m1[:, half, :, e0 : e0 + 512], ps[:], AF.Tanh
                )

            def gather_tail():
                # all types' and halves' 208-col tails: one psum, one ACT
                # call per half (2-dim free APs only; a 3-dim free out AP
                # misplaced the tail in an earlier attempt)
                pt = pA.tile(
                    [128, 2, NK, E - 2048], F32, tag="gather", name="ptail", bufs=2
                )
                for half in range(2):
                    for k in range(NK):
                        nc.tensor.matmul(
                            pt[:, half, k, :],
                            ABs[k][:, ts(half, 128)],
                            relT[:, 2048:E],
                        )
                for half in range(2):
                    nc.scalar.activation(
                        m1[:, half, :, 2048:E], pt[:, half], AF.Tanh
                    )

            def fc2(k, z2, groups=(0, NEC128)):
                for g0 in range(groups[0], groups[1], 2):
                    sub = EC128[g0 : g0 + 2]
                    ps = pB.tile([128, 2, H], F32, tag="ps")
                    for j, (e0, ew) in enumerate(sub):
                        for fc in range(2):
                            nc.tensor.matmul(
                                ps[:ew, j, :],
                                m1[:, fc, k, e0 : e0 + ew],
                                w2[:, fc, k, :],
                                start=(fc == 0),
                                stop=(fc == 1),
                            )
                    if sub[-1][1] == 128:
                        nc.vector.tensor_tensor(
                            z2[:, g0 : g0 + 2, :], ps[:], b2bc[:, k, :, :], ALU.add
                        )
                    else:
                        for j, (e0, ew) in enumerate(sub):
                            nc.vector.tensor_tensor(
                                z2[:ew, g0 + j, :],
                                ps[:ew, j, :],
                                b2bc[:ew, k, j, :],
                                ALU.add,
                            )

            def mega_tanh(k, parts=((0, NEC128),)):
                # full [128, chunks, H] strips; pad rows of the last chunk are
                # zeroed once at build, so tanh over them is clean (m2 pad
                # rows become 0, required by the fp8 scatter pairs).
                for c0, c1 in parts:
                    nc.scalar.activation(
                        m2s[k][:, c0:c1, :], z2s[k][:, c0:c1, :], AF.Tanh
                    )

            def scatter(k, agg_ps, first, last, chunks=(0, NEC128)):
                # single accumulation into rows 0:48 (all-fp8 runs at bf16
                # speed); no col packing so the aggregate drains with one
                # copy at the step boundary
                for ci in range(chunks[0], chunks[1]):
                    e0, ew = EC128[ci]
                    nc.tensor.matmul(
                        agg_ps[0:N, :],
                        wrec[:ew, k, ci, :],
                        m2s[k][:ew, ci, :],
                        start=(first and ci == 0),
                        stop=(last and ci == NEC128 - 1),
                        skip_group_check=True,
                    )

            def ka_warm():
                # HAM warmth filler: the GRU boundary leaves the PE mostly
                # idle for ~4us, which re-throttles the PE clock to 1.2 GHz
                # and makes every step's first ~3.4us of matmuls run at half
                # speed. These N=512 reads of constant w2 keep the activity
                # monitor busy through the boundary.
                kaw = pA.tile([128, 512], F32, tag="gather", name="kawarm", bufs=2)
                nc.tensor.matmul(kaw[0:2, :], w1[:, 0, 0, 0:2], w1[:, 0, 0, :])

            def edge_phase(pending_out=None):
                # Engine queues execute in order, so emission order IS the
                # schedule. The merge below rate-matches the PE against the
                # ACT m1-tanh stream: gathers of type k+1 and fc2 groups of
                # type k alternate at psum-tile granularity, fc2 of the last
                # type runs before the (mega-tanh-gated) scatters.
                node_fc1(0)
                node_fc1(1)
                node_fc1(2)
                gather_blk(0, 0)
                gather_blk(0, 1)
                if pending_out is not None:
                    pending_out()  # deferred out-MLP: filler during G0 pacing
                gather_blk(1, 0)
                gather_blk(1, 1)
                # fine-grained merge: one gather psum tile then fc2 pair
                # groups, so the PE returns to refill the m1-tanh pipe quickly
                # and the ACT stream never starves. fc2 chunks c need gather
                # block c//4 (both halves).
                fc2(0, z2s[0], (0, 4))
                gather_blk(2, 0)
                fc2(0, z2s[0], (4, 8))
                gather_blk(2, 1)
                fc2(1, z2s[1], (0, 4))
                gather_blk(3, 0)
                fc2(1, z2s[1], (4, 8))
                gather_blk(3, 1)
                fc2(0, z2s[0], (8, 12))
                gather_tail()
                fc2(1, z2s[1], (8, 12))
                fc2(0, z2s[0], (12, 16))
                fc2(1, z2s[1], (12, 16))
                fc2(0, z2s[0], (16, NEC128))
                fc2(1, z2s[1], (16, NEC128))
                mega_tanh(0)
                fc2(2, z2s[2], (0, 8))
                fc2(2, z2s[2], (8, 16))
                fc2(2, z2s[2], (16, NEC128))
                agg_ps = pA.tile([128, H], F32, tag="gather", name="aggps", bufs=2)
                scatter(0, agg_ps, True, False)
                mega_tanh(1)
                scatter(1, agg_ps, False, False)
                # last type is ACT-bound: interleave tanh parts with scatter
                # parts; last=True for every part since stop tags the final
                # accumulation MM of each col group (both in the final part)
                for c0, c1 in TANH_PARTS:
                    mega_tanh(2, parts=((c0, c1),))
                    scatter(2, agg_ps, False, True, chunks=(c0, c1))

                # aggregate: merge the two col groups and transpose to
                # feature-major, split per column half to shorten the chain
                agg_nm = wpool.tile([N, H], F32, tag="aggnm")
                nc.vector.tensor_copy(agg_nm[:], agg_ps[0:N, :])
                for half in range(2):
                    ka_warm()
                    tp = pB.tile([128, N], F32, tag="ps")
                    nc.tensor.transpose(tp[:], agg_nm[:, ts(half, 128)], ident[:N, :N])
                    nc.vector.tensor_copy(agg[:, half, :], tp[:])

            def gru_and_out(t):
                insT = ins5b[:, t, :]  # [5, 48]; row 4 is ones (folds input biases)
                # r gate first: it sits on the nng critical path; i only
                # matters at the final mix
                ps_r = pB.tile([128, 2, N], F32, tag="ps")
                for half in range(2):
                    nc.tensor.matmul(
                        ps_r[:, half, :], giw[:, 0, ts(half, 128)], insT,
                        start=True, stop=False,
                    )
                    for fc in range(2):
                        nc.tensor.matmul(
                            ps_r[:, half, :],
                            ghw[:, fc, 0, ts(half, 128)],
                            agg[:, fc, :],
                            start=False,
                            stop=(fc == 1),
                        )
                r_fm = wpool.tile([128, 2, N], F32, tag="gate0")
                nc.scalar.activation(r_fm[:], ps_r[:], AF.Sigmoid)
                ka_warm()
                ps_hn = pB.tile([128, 2, N], F32, tag="ps")
                ps_in = pB.tile([128, 2, N], F32, tag="ps")
                for half in range(2):
                    for fc in range(2):
                        nc.tensor.matmul(
                            ps_hn[:, half, :],
                            ghw[:, fc, 2, ts(half, 128)],
                            agg[:, fc, :],
                            start=(fc == 0),
                            stop=(fc == 1),
                        )
                    nc.tensor.matmul(
                        ps_in[:, half, :], giw[:, 2, ts(half, 128)], insT
                    )
                ka_warm()
                ps_i = pB.tile([128, 2, N], F32, tag="ps")
                for half in range(2):
                    nc.tensor.matmul(
                        ps_i[:, half, :], giw[:, 1, ts(half, 128)], insT,
                        start=True, stop=False,
                    )
                    for fc in range(2):
                        nc.tensor.matmul(
                            ps_i[:, half, :],
                            ghw[:, fc, 1, ts(half, 128)],
                            agg[:, fc, :],
                            start=False,
                            stop=(fc == 1),
                        )
                ka_warm()
                ka = pA.tile([128, N], F32, tag="gather", name="keepalive", bufs=2)
                nc.tensor.matmul(ka[0:2, :], r_fm[:, 0, 0:2], r_fm[:, 0, :])
                t1 = wpool.tile([128, 2, N], F32, tag="t1")
                nng = wpool.tile([128, 2, N], F32, tag="nng")
                dlt = wpool.tile([128, 2, N], F32, tag="dlt")
                i_fm = wpool.tile([128, 2, N], F32, tag="gate1")
                # ACT queue order matters: nng(h0) tanh is emitted BEFORE the
                # i-sigmoid so the critical path to hidden[h0] (and the next
                # step's fc1) doesn't wait behind it; both t1 halves go to
                # the DVE first so nng(h1) can fire right after.
                for h in range(2):
                    nc.vector.tensor_mul(t1[:, h, :], r_fm[:, h, :], ps_hn[:, h, :])
                    nc.vector.tensor_add(t1[:, h, :], t1[:, h, :], ps_in[:, h, :])
                nc.scalar.activation(nng[:, 0, :], t1[:, 0, :], AF.Tanh)
                nc.scalar.activation(i_fm[:], ps_i[:], AF.Sigmoid)
                nc.scalar.activation(nng[:, 1, :], t1[:, 1, :], AF.Tanh)
                # hidden = (1-i)*nng + i*hidden = nng + i*(hidden-nng),
                # per feature-half so hidden[:, h] lands early and the next
                # step's node-fc1 matmul can start sooner
                for h in range(2):
                    nc.vector.tensor_sub(dlt[:, h, :], hidden[:, h, :], nng[:, h, :])
                    nc.vector.tensor_mul(dlt[:, h, :], i_fm[:, h, :], dlt[:, h, :])
                    nc.vector.tensor_add(hidden[:, h, :], nng[:, h, :], dlt[:, h, :])
                nc.tensor.matmul(ka[0:2, :], nng[:, 0, 0:2], nng[:, 0, :])
                ka_warm()
                ka_warm()

                def out_mlp(t=t):
                    emit_out_mlp(t)
                return out_mlp

            def emit_out_mlp(t):
                # output MLP with residual
                cur = hidden
                for layer in range(2):
                    ps = pB.tile([128, 2, N], F32, tag="ps")
                    for half in range(2):
                        for fc in range(2):
                            nc.tensor.matmul(
                                ps[:, half, :],
                                ow12[:, fc, layer, ts(half, 128)],
                                cur[:, fc, :],
                                start=(fc == 0),
                                stop=(fc == 1),
                            )
                    nxt = wpool.tile([128, 2, N], BF16, tag=f"p{layer}")
                    for half in range(2):
                        nc.vector.tensor_scalar(
                            nxt[:, half, :],
                            ps[:, half, :],
                            ob12[:, half, layer : layer + 1],
                            0.0,
                            ALU.add,
                            ALU.max,
                        )
                    cur = nxt
                ps3 = pB.tile([D, N], F32, tag="ps")
                for fc in range(2):
                    nc.tensor.matmul(
                        ps3[:],
                        o3w[:, fc, :],
                        cur[:, fc, :],
                        start=(fc == 0),
                        stop=(fc == 1),
                    )
                # pred = (ps3 + b3) + ins
                nc.vector.scalar_tensor_tensor(
                    preds[:, t, :], ps3[:], o3b[:], ins5[0:D, t, :], ALU.add, ALU.add
                )
                nc.sync.dma_start(d_out[:, t, :], preds[:, t, :])

            pending_out = None
            for t in range(1 if skip_t0 else 0, TS):
                edge_phase(pending_out)
                pending_out = gru_and_out(t)
            pending_out()

    return nc


def _prep_core(b: int, inputs: dict) -> dict:
    f32 = np.float32
    bf16 = ml_dtypes.bfloat16
    fp8 = ml_dtypes.float8_e4m3
    data = np.asarray(inputs["data"], f32)
    rel_type = np.asarray(inputs["rel_type"], f32)
    rel_rec = np.asarray(inputs["rel_rec"], f32)
    rel_send = np.asarray(inputs["rel_send"], f32)
    w1 = np.asarray(inputs["msg_fc1_w"], f32)
    b1 = np.asarray(inputs["msg_fc1_b"], f32)
    w2 = np.asarray(inputs["msg_fc2_w"], f32)
    b2 = np.asarray(inputs["msg_fc2_b"], f32)

    m = {}
    dfm = np.ones((5, T, N), f32)
    dfm[0:4] = data[b].transpose(2, 1, 0)  # [N,T,D] -> [D,T,N]
    m["data_fm"] = dfm
    m["data_bf"] = dfm.astype(bf16)

    relT = np.zeros((128, E), f32)
    relT[0:N] = rel_rec.T
    relT[48] = 1.0  # bias row: pairs with AB row 48 = msg_fc1_b
    relT[64 : 64 + N] = rel_send.T
    m["relT"] = relT.astype(bf16)

    w1c = np.zeros((128, 2, NK, 2 * H), f32)
    for k in range(NK):
        wk = w1[k + 1]  # [2H, H]
        cat = np.concatenate([wk[:H], wk[H:]], axis=1)  # [H, 2H] = [W1r | W1s]
        w1c[:, :, k, :] = cat.reshape(2, 128, 2 * H).transpose(1, 0, 2)
    m["w1cat"] = w1c.astype(bf16)
    m["b1rows"] = np.stack([b1[k + 1] for k in range(NK)], axis=0).astype(bf16)

    m["w2"] = np.stack(
        [w2[k + 1].reshape(2, 128, H).transpose(1, 0, 2) for k in range(NK)], axis=2
    ).astype(bf16)
    b2bc = np.zeros((128, NK, 2, H), f32)
    for k in range(NK):
        b2bc[:, k, :, :] = b2[k + 1][None, None, :]
    m["b2bc"] = b2bc.astype(bf16)

    # wrec: [128, NK, NEC128, N] fp8, scaled x16 (1/16 folded into ghw)
    wr = np.zeros((128, NK, NEC128, N), f32)
    for k in range(NK):
        wk = rel_rec * rel_type[b, :, k + 1 : k + 2] * (WREC_SCALE / NORM)  # [E, N]
        wkp = np.zeros((NEC128 * 128, N), f32)
        wkp[:E] = wk
        wr[:, k] = wkp.reshape(NEC128, 128, N).transpose(1, 0, 2)
    m["wrec"] = wr.astype(fp8)

    m["gru_hw"] = (
        np.stack(
            [
                np.asarray(inputs[n], f32).reshape(2, 128, H).transpose(1, 0, 2)
                for n in ["hid_r_w", "hid_i_w", "hid_n_w"]
            ],
            axis=2,
        )
        / WREC_SCALE
    ).astype(bf16)
    giw = np.zeros((5, 3, H), f32)
    for g, (wn, bn) in enumerate(
        [("in_r_w", "in_r_b"), ("in_i_w", "in_i_b"), ("in_n_w", "in_n_b")]
    ):
        giw[0:4, g] = np.asarray(inputs[wn], f32)
        giw[4, g] = np.asarray(inputs[bn], f32)
    m["gru_iw"] = giw.astype(bf16)

    m["outw12"] = np.stack(
        [
            np.asarray(inputs[n], f32).reshape(2, 128, H).transpose(1, 0, 2)
            for n in ["out1_w", "out2_w"]
        ],
        axis=2,
    ).astype(bf16)
    m["outb12"] = np.stack(
        [np.asarray(inputs[n], f32).reshape(2, 128).T for n in ["out1_b", "out2_b"]],
        axis=2,
    )
    m["out3w"] = np.asarray(inputs["out3_w"], f32).reshape(2, 128, D).transpose(1, 0, 2).astype(bf16)
    m["out3b"] = np.asarray(inputs["out3_b"], f32).reshape(D, 1)
    return m


def _skip_t0_ok(inputs) -> bool:
    # With hidden0 == 0, the whole edge phase at t=0 yields agg == 0 iff the
    # message-MLP biases of the used edge types are zero.
    return bool(
        np.all(np.asarray(inputs["msg_fc1_b"])[1:] == 0)
        and np.all(np.asarray(inputs["msg_fc2_b"])[1:] == 0)
    )


def _host_t0(inputs):
    """Step 0 in exact fp32 on the host: hidden0 == 0 and zero message-MLP
    biases make agg(t=0) == 0, so only the GRU + output MLP run."""
    f32 = np.float32
    data = np.asarray(inputs["data"], f32)
    ins0 = data[:, :, 0, :]  # [B, N, D]
    sig = lambda x: 1.0 / (1.0 + np.exp(-x))
    i = sig(ins0 @ np.asarray(inputs["in_i_w"], f32) + np.asarray(inputs["in_i_b"], f32))
    n = np.tanh(ins0 @ np.asarray(inputs["in_n_w"], f32) + np.asarray(inputs["in_n_b"], f32))
    hidden1 = (1.0 - i) * n  # [B, N, H]
    p = np.maximum(hidden1 @ np.asarray(inputs["out1_w"], f32) + np.asarray(inputs["out1_b"], f32), 0)
    p = np.maximum(p @ np.asarray(inputs["out2_w"], f32) + np.asarray(inputs["out2_b"], f32), 0)
    pred0 = ins0 + p @ np.asarray(inputs["out3_w"], f32) + np.asarray(inputs["out3_b"], f32)
    return hidden1, pred0


def prepare(inputs):
    """Returns (skip_t0, in_maps, pred0) — shared by kernel() and harnesses."""
    skip_t0 = _skip_t0_ok(inputs)
    in_maps = [_prep_core(b, inputs) for b in range(B)]
    pred0 = None
    if skip_t0:
        hidden1, pred0 = _host_t0(inputs)
        for b in range(B):
            # hidden feature-major [128, 2, N]: [p, half, node] = hidden1[node, half*128+p]
            in_maps[b]["hid1"] = np.ascontiguousarray(
                hidden1[b].T.reshape(2, 128, N).transpose(1, 0, 2)
            ).astype(ml_dtypes.bfloat16)
    return skip_t0, in_maps, pred0


def kernel(**inputs) -> np.ndarray:
    assert int(inputs["pred_steps"]) == 1
    skip_t0, in_maps, pred0 = prepare(inputs)
    nc = build_decoder(skip_t0)
    nc.compile()
    res = run_bass_kernel_spmd(nc, in_maps, core_ids=list(range(B)))
    out = np.stack(
        [res.results[b]["preds"].transpose(2, 1, 0) for b in range(B)], axis=0
    )
    out = out.astype(np.float32)
    if skip_t0:
        out[:, :, 0, :] = pred0
    return out


if __name__ == "__main__":
    # smoke: build only
    nc = build_decoder(True)
    print("built ok")


# revision 47
# speedup vs baseline: 1.1827x; 1.1647x over previous
"""Trainium2 Bass kernel for the NRI decoder (gnn_message_passing).

Strategy: data-parallel over batch B=8 across the 8 NeuronCores (one batch
item per core, params replicated; zero collectives).

Per-core algorithm (per recurrent step t, 9 steps):
  - fc1 of the edge MLP is factored through the nodes (exact, by
    associativity): pre @ W1 = rel_rec @ (hidden @ W1r) + rel_send @ (hidden @ W1s)
    so the heavy per-edge fc1 matmul collapses to two node-level matmuls
    (A = hidden@W1r, B = hidden@W1s) plus gather matmuls with rel_rec/rel_send.
  - gathers, fc2 and the scatter-aggregate are plain matmuls (no one-hot
    assumption anywhere), with the edge-type weights rtyp/(norm*d) folded into
    precomputed scatter weights wrec_k = rel_rec * rel_type[:, k] / 12 so the
    weighted sum over edge types becomes PSUM accumulation.
  - the scatter runs in fp8e4 with DoubleRow perf mode: chunk-pairs are
    fused into single 256-contraction matmuls (wrec pre-scaled x16 for fp8
    range; the 1/16 is folded into the GRU hid weights).
  - feature-major layouts chain all matmuls without transposes except one
    tiny [48,256] transpose of the aggregate per step.

Self-contained: hardcodes all shapes; no file reads.
"""

import numpy as np
import ml_dtypes

import concourse.tile as tile
from concourse import bacc, mybir
from concourse.bass import ts
from concourse.bass_utils import run_bass_kernel_spmd
from concourse.masks import make_identity

# Problem constants
B, N, T, D, H, K = 8, 48, 10, 4, 256, 4
E = N * (N - 1)          # 2256
NK = K - 1               # 3 used edge types (type 0 skipped)
TS = T - 1               # 9 recurrent steps
NORM = float(NK * D)     # combined 1/(K-1) and 1/n_in_node scaling
WREC_SCALE = 16.0        # fp8 range scaling for wrec; 1/16 folded into ghw

F32 = mybir.dt.float32
BF16 = mybir.dt.bfloat16
FP8 = mybir.dt.float8e4
AF = mybir.ActivationFunctionType
ALU = mybir.AluOpType
DR = mybir.MatmulPerfMode.DoubleRow

NEC128 = (E + 127) // 128                                   # 18
NPAIR = NEC128 // 2                                         # 9 chunk pairs
EC128 = [(i * 128, min(128, E - i * 128)) for i in range(NEC128)]
# tanh/scatter interleave parts for the last edge type
TANH_PARTS = [(0, 9), (9, NEC128)]


def build_decoder(skip_t0: bool):
    nc = bacc.Bacc("TRN2", target_bir_lowering=False)

    d_data = nc.dram_tensor("data_fm", [5, T, N], F32, kind="ExternalInput")
    d_data_bf = nc.dram_tensor("data_bf", [5, T, N], BF16, kind="ExternalInput")
    d_relT = nc.dram_tensor("relT", [128, E], BF16, kind="ExternalInput")
    d_w1 = nc.dram_tensor("w1cat", [128, 2, NK, 2 * H], BF16, kind="ExternalInput")
    d_b1r = nc.dram_tensor("b1rows", [NK, H], BF16, kind="ExternalInput")
    d_w2 = nc.dram_tensor("w2", [128, 2, NK, H], BF16, kind="ExternalInput")
    d_b2 = nc.dram_tensor("b2bc", [128, NK, 2, H], BF16, kind="ExternalInput")
    d_wrec = nc.dram_tensor("wrec", [128, NK, NEC128, N], FP8, kind="ExternalInput")
    d_ghw = nc.dram_tensor("gru_hw", [128, 2, 3, H], BF16, kind="ExternalInput")
    d_giw = nc.dram_tensor("gru_iw", [5, 3, H], BF16, kind="ExternalInput")
    d_ow12 = nc.dram_tensor("outw12", [128, 2, 2, H], BF16, kind="ExternalInput")
    d_ob12 = nc.dram_tensor("outb12", [128, 2, 2], F32, kind="ExternalInput")
    d_o3w = nc.dram_tensor("out3w", [128, 2, D], BF16, kind="ExternalInput")
    d_o3b = nc.dram_tensor("out3b", [D, 1], F32, kind="ExternalInput")
    d_hid1 = None
    if skip_t0:
        d_hid1 = nc.dram_tensor("hid1", [128, 2, N], BF16, kind="ExternalInput")
    d_out = nc.dram_tensor("preds", [D, TS, N], F32, kind="ExternalOutput")

    with tile.TileContext(nc) as tc:
        with (
            tc.tile_pool(name="const", bufs=1) as cpool,
            tc.tile_pool(name="state", bufs=1) as spool,
            tc.tile_pool(name="work", bufs=3) as wpool,
            tc.tile_pool(name="stage", bufs=1) as zpool,
            tc.tile_pool(name="pA", bufs=1, space="PSUM") as pA,
            tc.tile_pool(name="pB", bufs=3, space="PSUM") as pB,
            tc.tile_pool(name="pC", bufs=1, space="PSUM") as pC,
        ):
            # ---------------- constants ----------------
            # Startup DMAs split across the three queues in consumption
            # order, pieced so the first gather can start ~3us in:
            #   sync:   hidden, w1 per edge type (fc1 path)
            #   scalar: relT in column pieces, then w2 (gather/fc2 path)
            #   gpsimd: everything else
            hidden = spool.tile([128, 2, N], BF16)
            if skip_t0:
                nc.sync.dma_start(hidden[:], d_hid1[:])
            else:
                nc.vector.memset(hidden[:], 0.0)
            w1 = cpool.tile([128, 2, NK, 2 * H], BF16)
            for k in range(NK):
                nc.sync.dma_start(w1[:, :, k, :], d_w1[:, :, k, :])
            b2bc = cpool.tile([128, NK, 2, H], BF16)
            nc.sync.dma_start(b2bc[:], d_b2[:])
            relT = cpool.tile([128, E], BF16)
            for e0, e1 in ((0, 512), (512, 1024), (1024, 2048), (2048, E)):
                nc.scalar.dma_start(relT[:, e0:e1], d_relT[:, e0:e1])
            w2 = cpool.tile([128, 2, NK, H], BF16)
            nc.scalar.dma_start(w2[:], d_w2[:])
            wrec = cpool.tile([128, NK, NEC128, N], FP8)
            nc.scalar.dma_start(wrec[:], d_wrec[:])
            ins5b = spool.tile([5, T, N], BF16)
            nc.gpsimd.dma_start(ins5b[:], d_data_bf[:])
            ghw = cpool.tile([128, 2, 3, H], BF16)
            nc.gpsimd.dma_start(ghw[:], d_ghw[:])
            giw = cpool.tile([5, 3, H], BF16)
            nc.gpsimd.dma_start(giw[:], d_giw[:])
            ow12 = cpool.tile([128, 2, 2, H], BF16)
            nc.gpsimd.dma_start(ow12[:], d_ow12[:])
            ob12 = cpool.tile([128, 2, 2], F32)
            nc.gpsimd.dma_start(ob12[:], d_ob12[:])
            o3w = cpool.tile([128, 2, D], BF16)
            nc.gpsimd.dma_start(o3w[:], d_o3w[:])
            o3b = cpool.tile([D, 1], F32)
            nc.gpsimd.dma_start(o3b[:], d_o3b[:])
            ins5 = spool.tile([5, T, N], F32)
            nc.gpsimd.dma_start(ins5[:], d_data[:])
            ident = cpool.tile([128, 128], F32)
            make_identity(nc, ident[:])

            # ---------------- state ----------------
            agg = spool.tile([128, 2, N], BF16)
            nc.vector.memset(agg[:], 0.0)
            preds = spool.tile([D, TS, N], F32)
            m1 = spool.tile([128, 2, NK, E], BF16)
            # AB_k: rows 0:48 = A = hidden@W1r (node-major), rows 64:112 = B.
            # Rows 48:64 / 112:128 stay zero forever (they hit zero rows of relT).
            ABs = []
            for k in range(NK):
                ab = spool.tile([128, H], BF16, tag=f"AB{k}")
                nc.vector.memset(ab[:], 0.0)
                nc.sync.dma_start(ab[48:49, :], d_b1r[k : k + 1, :])
                ABs.append(ab)
            # z2/m2 strips live across steps; the pad rows of the last chunk
            # are zeroed once so full-strip tanh + fp8 scatter read clean data.
            z2s = [
                zpool.tile([128, NEC128, H], F32, tag=f"z2_{k}", name=f"z2_{k}")
                for k in range(NK)
            ]
            m2s = [
                zpool.tile([128, NEC128, H], FP8, tag=f"m2_{k}", name=f"m2_{k}")
                for k in range(NK)
            ]
            # pad-zero the last chunk's tail rows (base partition must be
            # 32-aligned; rows 64:80 get overwritten by real data each step)
            for k in range(NK):
                nc.vector.memset(z2s[k][64:128, NEC128 - 1, :], 0.0)

            def node_fc1(k):
                ps = pB.tile([N, 2 * H], F32, tag="ps")
                for fc in range(2):
                    nc.tensor.matmul(
                        ps[:],
                        hidden[:, fc, :],
                        w1[:, fc, k, :],
                        start=(fc == 0),
                        stop=(fc == 1),
                    )
                # col-half-split copies so the first gather LDW (cols 0:128
                # of both A and B) unblocks half a copy earlier
                for c in range(2):
                    nc.vector.tensor_copy(
                        ABs[k][0:N, ts(c, 128)], ps[:, ts(c, 128)]
                    )
                    nc.vector.tensor_copy(
                        ABs[k][64 : 64 + N, ts(c, 128)], ps[:, H + 128 * c : H + 128 * (c + 1)]
                    )

            def gather_blk(k, blk, half):
                # m1[h', e] = tanh(A[recv] + B[send] + b1): one matmul on the
                # stacked [A;B] / [rel_rec.T; rel_send.T] operands; the fc1
                # bias rides contract-row 48 (relT row 48 is all-ones, AB row
                # 48 holds b1), so the tanh needs no bias operand.
                e0 = blk * 1024
                ps = pA.tile([128, 1024], F32, tag="gather", bufs=2)
                for c0 in range(0, 1024, 512):
                    nc.tensor.matmul(
                        ps[:, c0 : c0 + 512],
                        ABs[k][:, ts(half, 128)],
                        relT[:, e0 + c0 : e0 + c0 + 512],
                    )
                nc.scalar.activation(
                    m1[:, half, k, e0 : e0 + 1024], ps[:], AF.Tanh
                )

            def gather_tail(k):
                # both halves' 208-col tails share one psum + one ACT
                pt = pA.tile([128, 2, E - 2048], F32, tag="gather", name="ptail", bufs=2)
                for half in range(2):
                    nc.tensor.matmul(
                        pt[:, half, :],
                        ABs[k][:, ts(half, 128)],
                        relT[:, 2048:E],
                    )
                nc.scalar.activation(m1[:, 0:2, k, 2048:E], pt[:], AF.Tanh)

            def fc2(k, z2, groups=(0, NEC128)):
                for g0 in range(groups[0], groups[1], 2):
                    sub = EC128[g0 : g0 + 2]
                    ps = pB.tile([128, 2, H], F32, tag="ps")
                    for j, (e0, ew) in enumerate(sub):
                        for fc in range(2):
                            nc.tensor.matmul(
                                ps[:ew, j, :],
                                m1[:, fc, k, e0 : e0 + ew],
                                w2[:, fc, k, :],
                                start=(fc == 0),
                                stop=(fc == 1),
                            )
                    if sub[-1][1] == 128:
                        nc.vector.tensor_tensor(
                            z2[:, g0 : g0 + 2, :], ps[:], b2bc[:, k, :, :], ALU.add
                        )
                    else:
                        for j, (e0, ew) in enumerate(sub):
                            nc.vector.tensor_tensor(
                                z2[:ew, g0 + j, :],
                                ps[:ew, j, :],
                                b2bc[:ew, k, j, :],
                                ALU.add,
                            )

            def mega_tanh(k, parts=((0, NEC128),)):
                # full [128, chunks, H] strips; pad rows of the last chunk are
                # zeroed once at build, so tanh over them is clean (m2 pad
                # rows become 0, required by the fp8 scatter pairs).
                for c0, c1 in parts:
                    nc.scalar.activation(
                        m2s[k][:, c0:c1, :], z2s[k][:, c0:c1, :], AF.Tanh
                    )

            def scatter(k, agg_ps, first, last, chunks=(0, NEC128)):
                # single accumulation into rows 0:48 (all-fp8 runs at bf16
                # speed); no col packing so the aggregate drains with one
                # copy at the step boundary
                for ci in range(chunks[0], chunks[1]):
                    e0, ew = EC128[ci]
                    nc.tensor.matmul(
                        agg_ps[0:N, :],
                        wrec[:ew, k, ci, :],
                        m2s[k][:ew, ci, :],
                        start=(first and ci == 0),
                        stop=(last and ci == NEC128 - 1),
                        skip_group_check=True,
                    )

            def ka_warm():
                # HAM warmth filler: the GRU boundary leaves the PE mostly
                # idle for ~4us, which re-throttles the PE clock to 1.2 GHz
                # and makes every step's first ~3.4us of matmuls run at half
                # speed. These N=512 reads of constant w2 keep the activity
                # monitor busy through the boundary.
                kaw = pA.tile([128, 512], F32, tag="gather", name="kawarm", bufs=2)
                nc.tensor.matmul(kaw[0:2, :], w1[:, 0, 0, 0:2], w1[:, 0, 0, :])

            def edge_phase(pending_out=None):
                # Engine queues execute in order, so emission order IS the
                # schedule. The merge below rate-matches the PE against the
                # ACT m1-tanh stream: gathers of type k+1 and fc2 groups of
                # type k alternate at psum-tile granularity, fc2 of the last
                # type runs before the (mega-tanh-gated) scatters.
                node_fc1(0)
                node_fc1(1)
                node_fc1(2)
                gather_blk(0, 0, 0)
                gather_blk(0, 0, 1)
                if pending_out is not None:
                    pending_out()  # deferred out-MLP: filler during G0 pacing
                gather_blk(0, 1, 0)
                gather_blk(0, 1, 1)
                gather_tail(0)
                # fine-grained merge: one gather psum tile then one fc2 pair
                # group, so the PE returns to refill the m1-tanh pipe quickly
                # and the ACT stream never starves
                gather_blk(1, 0, 0)
                fc2(0, z2s[0], (0, 2))
                gather_blk(1, 0, 1)
                fc2(0, z2s[0], (2, 4))
                gather_blk(1, 1, 0)
                fc2(0, z2s[0], (4, 6))
                gather_blk(1, 1, 1)
                fc2(0, z2s[0], (6, 8))
                gather_tail(1)
                fc2(0, z2s[0], (8, 12))
                gather_blk(2, 0, 0)
                fc2(0, z2s[0], (12, 16))
                gather_blk(2, 0, 1)
                fc2(0, z2s[0], (16, NEC128))
                gather_blk(2, 1, 0)
                fc2(1, z2s[1], (0, 2))
                gather_blk(2, 1, 1)
                fc2(1, z2s[1], (2, 4))
                gather_tail(2)
                fc2(1, z2s[1], (4, 8))
                fc2(1, z2s[1], (8, 12))
                fc2(1, z2s[1], (12, 16))
                fc2(1, z2s[1], (16, NEC128))
                mega_tanh(0)
                fc2(2, z2s[2], (0, 8))
                fc2(2, z2s[2], (8, 16))
                fc2(2, z2s[2], (16, NEC128))
                agg_ps = pC.tile([128, H], F32, tag="agg")
                scatter(0, agg_ps, True, False)
                mega_tanh(1)
                scatter(1, agg_ps, False, False)
                # last type is ACT-bound: interleave tanh parts with scatter
                # parts; last=True for every part since stop tags the final
                # accumulation MM of each col group (both in the final part)
                for c0, c1 in TANH_PARTS:
                    mega_tanh(2, parts=((c0, c1),))
                    scatter(2, agg_ps, False, True, chunks=(c0, c1))

                # aggregate: merge the two col groups and transpose to
                # feature-major, split per column half to shorten the chain
                agg_nm = wpool.tile([N, H], F32, tag="aggnm")
                nc.vector.tensor_copy(agg_nm[:], agg_ps[0:N, :])
                for half in range(2):
                    ka_warm()
                    tp = pB.tile([128, N], F32, tag="ps")
                    nc.tensor.transpose(tp[:], agg_nm[:, ts(half, 128)], ident[:N, :N])
                    nc.vector.tensor_copy(agg[:, half, :], tp[:])

            def gru_and_out(t):
                insT = ins5b[:, t, :]  # [5, 48]; row 4 is ones (folds input biases)
                # r gate first: it sits on the nng critical path; i only
                # matters at the final mix
                ps_r = pB.tile([128, 2, N], F32, tag="ps")
                for half in range(2):
                    nc.tensor.matmul(
                        ps_r[:, half, :], giw[:, 0, ts(half, 128)], insT,
                        start=True, stop=False,
                    )
                    for fc in range(2):
                        nc.tensor.matmul(
                            ps_r[:, half, :],
                            ghw[:, fc, 0, ts(half, 128)],
                            agg[:, fc, :],
                            start=False,
                            stop=(fc == 1),
                        )
                r_fm = wpool.tile([128, 2, N], F32, tag="gate0")
                nc.scalar.activation(r_fm[:], ps_r[:], AF.Sigmoid)
                ka_warm()
                ps_hn = pB.tile([128, 2, N], F32, tag="ps")
                ps_in = pB.tile([128, 2, N], F32, tag="ps")
                for half in range(2):
                    for fc in range(2):
                        nc.tensor.matmul(
                            ps_hn[:, half, :],
                            ghw[:, fc, 2, ts(half, 128)],
                            agg[:, fc, :],
                            start=(fc == 0),
                            stop=(fc == 1),
                        )
                    nc.tensor.matmul(
                        ps_in[:, half, :], giw[:, 2, ts(half, 128)], insT
                    )
                ka_warm()
                ps_i = pB.tile([128, 2, N], F32, tag="ps")
                for half in range(2):
                    nc.tensor.matmul(
                        ps_i[:, half, :], giw[:, 1, ts(half, 128)], insT,
                        start=True, stop=False,
                    )
                    for fc in range(2):
                        nc.tensor.matmul(
                            ps_i[:, half, :],
                            ghw[:, fc, 1, ts(half, 128)],
                            agg[:, fc, :],
                            start=False,
                            stop=(fc == 1),
                        )
                ka_warm()
                ka = pA.tile([128, N], F32, tag="gather", name="keepalive", bufs=2)
                nc.tensor.matmul(ka[0:2, :], r_fm[:, 0, 0:2], r_fm[:, 0, :])
                t1 = wpool.tile([128, 2, N], F32, tag="t1")
                nng = wpool.tile([128, 2, N], F32, tag="nng")
                dlt = wpool.tile([128, 2, N], F32, tag="dlt")
                i_fm = wpool.tile([128, 2, N], F32, tag="gate1")
                # ACT queue order matters: nng(h0) tanh is emitted BEFORE the
                # i-sigmoid so the critical path to hidden[h0] (and the next
                # step's fc1) doesn't wait behind it; both t1 halves go to
                # the DVE first so nng(h1) can fire right after.
                for h in range(2):
                    nc.vector.tensor_mul(t1[:, h, :], r_fm[:, h, :], ps_hn[:, h, :])
                    nc.vector.tensor_add(t1[:, h, :], t1[:, h, :], ps_in[:, h, :])
                nc.scalar.activation(nng[:, 0, :], t1[:, 0, :], AF.Tanh)
                nc.scalar.activation(i_fm[:], ps_i[:], AF.Sigmoid)
                nc.scalar.activation(nng[:, 1, :], t1[:, 1, :], AF.Tanh)
                # hidden = (1-i)*nng + i*hidden = nng + i*(hidden-nng),
                # per feature-half so hidden[:, h] lands early and the next
                # step's node-fc1 matmul can start sooner
                for h in range(2):
                    nc.vector.tensor_sub(dlt[:, h, :], hidden[:, h, :], nng[:, h, :])
                    nc.vector.tensor_mul(dlt[:, h, :], i_fm[:, h, :], dlt[:, h, :])
                    nc.vector.tensor_add(hidden[:, h, :], nng[:, h, :], dlt[:, h, :])
                nc.tensor.matmul(ka[0:2, :], nng[:, 0, 0:2], nng[:, 0, :])
                ka_warm()
                ka_warm()

                def out_mlp(t=t):
                    emit_out_mlp(t)
                return out_mlp

            def emit_out_mlp(t):
                # output MLP with residual
                cur = hidden
                for layer in range(2):
                    ps = pB.tile([128, 2, N], F32, tag="ps")
                    for half in range(2):
                        for fc in range(2):
                            nc.tensor.matmul(
                                ps[:, half, :],
                                ow12[:, fc, layer, ts(half, 128)],
                                cur[:, fc, :],
                                start=(fc == 0),
                                stop=(fc == 1),
                            )
                    nxt = wpool.tile([128, 2, N], BF16, tag=f"p{layer}")
                    for half in range(2):
                        nc.vector.tensor_scalar(
                            nxt[:, half, :],
                            ps[:, half, :],
                            ob12[:, half, layer : layer + 1],
                            0.0,
                            ALU.add,
                            ALU.max,
                        )
                    cur = nxt
                ps3 = pB.tile([D, N], F32, tag="ps")
                for fc in range(2):
                    nc.tensor.matmul(
                        ps3[:],
                        o3w[:, fc, :],
                        cur[:, fc, :],
                        start=(fc == 0),
                        stop=(fc == 1),
                    )
                # pred = (ps3 + b3) + ins
                nc.vector.scalar_tensor_tensor(
                    preds[:, t, :], ps3[:], o3b[:], ins5[0:D, t, :], ALU.add, ALU.add
                )
                nc.sync.dma_start(d_out[:, t, :], preds[:, t, :])

            pending_out = None
            for t in range(1 if skip_t0 else 0, TS):
                edge_phase(pending_out)
                pending_out = gru_and_out(t)
            pending_out()

    return nc


def _prep_core(b: int, inputs: dict) -> dict:
    f32 = np.float32
    bf16 = ml_dtypes.bfloat16
    fp8 = ml_dtypes.float8_e4m3
    data = np.asarray(inputs["data"], f32)
    rel_type = np.asarray(inputs["rel_type"], f32)
    rel_rec = np.asarray(inputs["rel_rec"], f32)
    rel_send = np.asarray(inputs["rel_send"], f32)
    w1 = np.asarray(inputs["msg_fc1_w"], f32)
    b1 = np.asarray(inputs["msg_fc1_b"], f32)
    w2 = np.asarray(inputs["msg_fc2_w"], f32)
    b2 = np.asarray(inputs["msg_fc2_b"], f32)

    m = {}
    dfm = np.ones((5, T, N), f32)
    dfm[0:4] = data[b].transpose(2, 1, 0)  # [N,T,D] -> [D,T,N]
    m["data_fm"] = dfm
    m["data_bf"] = dfm.astype(bf16)

    relT = np.zeros((128, E), f32)
    relT[0:N] = rel_rec.T
    relT[48] = 1.0  # bias row: pairs with AB row 48 = msg_fc1_b
    relT[64 : 64 + N] = rel_send.T
    m["relT"] = relT.astype(bf16)

    w1c = np.zeros((128, 2, NK, 2 * H), f32)
    for k in range(NK):
        wk = w1[k + 1]  # [2H, H]
        cat = np.concatenate([wk[:H], wk[H:]], axis=1)  # [H, 2H] = [W1r | W1s]
        w1c[:, :, k, :] = cat.reshape(2, 128, 2 * H).transpose(1, 0, 2)
    m["w1cat"] = w1c.astype(bf16)
    m["b1rows"] = np.stack([b1[k + 1] for k in range(NK)], axis=0).astype(bf16)

    m["w2"] = np.stack(
        [w2[k + 1].reshape(2, 128, H).transpose(1, 0, 2) for k in range(NK)], axis=2
    ).astype(bf16)
    b2bc = np.zeros((128, NK, 2, H), f32)
    for k in range(NK):
        b2bc[:, k, :, :] = b2[k + 1][None, None, :]
    m["b2bc"] = b2bc.astype(bf16)

    # wrec: [128, NK, NEC128, N] fp8, scaled x16 (1/16 folded into ghw)
    wr = np.zeros((128, NK, NEC128, N), f32)
    for k in range(NK):
        wk = rel_rec * rel_type[b, :, k + 1 : k + 2] * (WREC_SCALE / NORM)  # [E, N]
        wkp = np.zeros((NEC128 * 128, N), f32)
        wkp[:E] = wk
        wr[:, k] = wkp.reshape(NEC128, 128, N).transpose(1, 0, 2)
    m["wrec"] = wr.astype(fp8)

    m["gru_hw"] = (
        np.stack(
            [
                np.asarray(inputs[n], f32).reshape(2, 128, H).transpose(1, 0, 2)
                for n in ["hid_r_w", "hid_i_w", "hid_n_w"]
            ],
            axis=2,
        )
        / WREC_SCALE
    ).astype(bf16)
    giw = np.zeros((5, 3, H), f32)
    for g, (wn, bn) in enumerate(
        [("in_r_w", "in_r_b"), ("in_i_w", "in_i_b"), ("in_n_w", "in_n_b")]
    ):
        giw[0:4, g] = np.asarray(inputs[wn], f32)
        giw[4, g] = np.asarray(inputs[bn], f32)
    m["gru_iw"] = giw.astype(bf16)

    m["outw12"] = np.stack(
        [
            np.asarray(inputs[n], f32).reshape(2, 128, H).transpose(1, 0, 2)
            for n in ["out1_w", "out2_w"]
        ],
        axis=2,
    ).astype(bf16)
    m["outb12"] = np.stack(
        [np.asarray(inputs[n], f32).reshape(2, 128).T for n in ["out1_b", "out2_b"]],
        axis=2,
    )
    m["out3w"] = np.asarray(inputs["out3_w"], f32).reshape(2, 128, D).transpose(1, 0, 2).astype(bf16)
    m["out3b"] = np.asarray(inputs["out3_b"], f32).reshape(D, 1)
    return m


def _skip_t0_ok(inputs) -> bool:
    # With hidden0 == 0, the whole edge phase at t=0 yields agg == 0 iff the
    # message-MLP biases of the used edge types are zero.
    return bool(
        np.all(np.asarray(inputs["msg_fc1_b"])[1:] == 0)
        and np.all(np.asarray(inputs["msg_fc2_b"])[1:] == 0)
    )


def _host_t0(inputs):
    """Step 0 in exact fp32 on the host: hidden0 == 0 and zero message-MLP
    biases make agg(t=0) == 0, so only the GRU + output MLP run."""
    f32 = np.float32
    data = np.asarray(inputs["data"], f32)
    ins0 = data[:, :, 0, :]  # [B, N, D]
    sig = lambda x: 1.0 / (1.0 + np.exp(-x))
    i = sig(ins0 @ np.asarray(inputs["in_i_w"], f32) + np.asarray(inputs["in_i_b"], f32))
    n = np.tanh(ins0 @ np.asarray(inputs["in_n_w"], f32) + np.asarray(inputs["in_n_b"], f32))
    hidden1 = (1.0 - i) * n  # [B, N, H]
    p = np.maximum(hidden1 @ np.asarray(inputs["out1_w"], f32) + np.asarray(inputs["out1_b"], f32), 0)
    p = np.maximum(p @ np.asarray(inputs["out2_w"], f32) + np.asarray(inputs["out2_b"], f32), 0)
    pred0 = ins0 + p @ np.asarray(inputs["out3_w"], f32) + np.asarray(inputs["out3_b"], f32)
    return hidden1, pred0


def prepare(inputs):
    """Returns (skip_t0, in_maps, pred0) — shared by kernel() and harnesses."""
    skip_t0 = _skip_t0_ok(inputs)
    in_maps = [_prep_core(b, inputs) for b in range(B)]
    pred0 = None
    if skip_t0:
        hidden1, pred0 = _host_t0(inputs)
        for b in range(B):
            # hidden feature-major [128, 2, N]: [p, half, node] = hidden1[node, half*128+p]
            in_maps[b]["hid1"] = np.ascontiguousarray(
                hidden1[b].T.reshape(2, 128, N).transpose(1, 0, 2)
            ).astype(ml_dtypes.bfloat16)
    return skip_t0, in_maps, pred0


def kernel(**inputs) -> np.ndarray:
    assert int(inputs["pred_steps"]) == 1
    skip_t0, in_maps, pred0 = prepare(inputs)
    nc = build_decoder(skip_t0)
    nc.compile()
    res = run_bass_kernel_spmd(nc, in_maps, core_ids=list(range(B)))
    out = np.stack(
        [res.results[b]["preds"].transpose(2, 1, 0) for b in range(B)], axis=0
    )
    out = out.astype(np.float32)
    if skip_t0:
        out[:, :, 0, :] = pred0
    return out


if __name__ == "__main__":
    # smoke: build only
    nc = build_decoder(True)
    print("built ok")
